# revision 36
# baseline (speedup 1.0000x reference)
"""AttentionBlock Trainium2 kernel — 8-core SPMD, bf16 matmul path.

Sharding: core c -> batch b=c//4, head-pair g=c%4 (heads 2g, 2g+1).
Per core: LN1(all 2048 rows of batch b) -> qkv proj for its 2 heads ->
attention -> per-head merge-proj partials (bm/4 folded in) ->
ReduceScatter(+, bf16) within the 4-core batch group per 512-row
q-block -> each core owns 4x128 rows of x2 -> LN2 + FF(Swish) +
residual -> output chunk [512, 512] f32.

v2 changes vs v1:
- all matmuls bf16 (FWL halves weight-load overhead; psum stays f32)
- softmax denominator via ones[128,128] stationary matmul accumulation
  -> full-width [128,512] reciprocal (was [1,512] single-lane, 6us)
- weights prefetched on the gpsimd DMA queue (x stream on sync queue)
- ReduceScatter payload bf16 (half the collective time)
- phase C emitted per-qb interleaved with phase B; FF1 split in halves
  so only the last quarter of FF work sits behind the final RS
"""

import numpy as np
import ml_dtypes
import concourse.bass as bass
import concourse.bacc as bacc
import concourse.mybir as mybir
import concourse.tile as tile
from concourse import bass_utils
from concourse.masks import make_identity

P = 128
N = 2048          # sequence length
D = 512           # d_in / d_out
H2 = 2            # heads per core
DH = 64           # head dim (q, k)
DV = 512          # per-head value dim
E = 2048          # ff expand
QB = 512          # query block
NQB = N // QB     # 4
NRT = N // P      # 16 row tiles
KC = D // P       # 4 contraction chunks of d_in
EC = E // P       # 16 contraction chunks of d_expand
EPS = 1e-5
SCALE = DH ** -0.5

f32 = mybir.dt.float32
bf16 = mybir.dt.bfloat16
f8 = mybir.dt.float8e4
DR = mybir.MatmulPerfMode.DoubleRow
LN16 = 2.772588722239781  # exp(s - ln16): e4m3 saturates at 448, max s ~7.6

AF = mybir.ActivationFunctionType
ALU = mybir.AluOpType


def bcast_ap(ap, parts, free):
    """Partition-broadcast read AP for a [1, free] DRAM tensor."""
    return bass.AP(tensor=ap.tensor, offset=ap.offset, ap=[[0, parts], [1, free]])


def build_body(tc, ins, outs):
    nc = tc.nc
    (x, xr, wqkv, bqk_pt_d, bv_d, wm, bm4_d, w1, b1_pt_d, w2, b2_d) = ins
    out = outs["out"]

    import contextlib
    est = contextlib.ExitStack()
    with est:
        const = est.enter_context(tc.tile_pool(name="const", bufs=1))
        dram = est.enter_context(tc.tile_pool(name="dram", bufs=1, space="DRAM"))

        ident_f = const.tile([P, P], f32)
        make_identity(nc, ident_f)
        ident = const.tile([P, P], bf16)
        nc.vector.tensor_copy(ident, ident_f)
        ones2_f8 = const.tile([P, 2, 16], f8)
        nc.vector.memset(ones2_f8, 1.0)
        eps_t = const.tile([P, 1], f32)
        nc.vector.memset(eps_t, EPS)
        ln4_t = const.tile([P, 1], f32)
        nc.vector.memset(ln4_t, -LN16)

        # small constants + residual rows on the gpsimd DMA queue
        bqk_pt = const.tile([P, 2], f32)
        nc.gpsimd.dma_start(out=bqk_pt, in_=bqk_pt_d[:, :])
        bv_b = const.tile([P, 2, DV], f32)
        nc.gpsimd.dma_start(out=bv_b, in_=bcast_ap(bv_d, P, 2 * DV))
        bm4_b = const.tile([P, D], f32)
        nc.gpsimd.dma_start(out=bm4_b, in_=bcast_ap(bm4_d, P, D))
        b1_pt = const.tile([P, EC], f32)
        nc.gpsimd.dma_start(out=b1_pt, in_=b1_pt_d[:, :])
        b2_b = const.tile([P, D], f32)
        nc.gpsimd.dma_start(out=b2_b, in_=bcast_ap(b2_d, P, D))

        # persistent weights (prefetched early, gpsimd queue)
        poolW = est.enter_context(tc.tile_pool(name="poolW", bufs=1))
        wqkv_sb = poolW.tile([P, KC, 2 * H2 * DH + H2 * DV], bf16)
        wqkv_r = wqkv.rearrange("(c p) n -> p c n", p=P)
        for kc in range(KC):
            nc.gpsimd.dma_start(out=wqkv_sb[:, kc, :], in_=wqkv_r[:, kc, :])
        wm_sb = poolW.tile([P, H2 * DV // P, D], bf16)
        nc.gpsimd.dma_start(out=wm_sb, in_=wm.rearrange("(c p) n -> p c n", p=P))
        xr_sb = poolW.tile([P, NQB, D], f32)
        nc.gpsimd.dma_start(out=xr_sb, in_=xr.rearrange("q p d -> p q d"))
        w1_sb = poolW.tile([P, KC, E], bf16)
        w1r = w1.rearrange("(c p) n -> p c n", p=P)
        for kc in range(KC):
            nc.gpsimd.dma_start(out=w1_sb[:, kc, :], in_=w1r[:, kc, :])
        w2_sb = poolW.tile([P, EC, D], bf16)
        w2r = w2.rearrange("(c p) n -> p c n", p=P)
        for j in range(4):
            nc.gpsimd.dma_start(out=w2_sb[:, 4 * j:4 * (j + 1), :],
                                in_=w2r[:, 4 * j:4 * (j + 1), :])

        # DRAM bounce buffers for the ReduceScatter (bf16, one per q-block)
        rs_in = [dram.tile([QB, D], bf16, name=f"rs_in{j}", tag=f"rs_in{j}")
                 for j in range(NQB)]
        rs_out = [dram.tile([P, D], bf16, name=f"rs_out{j}", tag=f"rs_out{j}")
                  for j in range(NQB)]

        # Dummy tiny collective, first in the CC pipeline: absorbs cross-core
        # launch skew (~30us peer-wait otherwise paid by RS(0), delaying the
        # whole serialized collective spine) while the PE runs phase A.
        # (Collectives cannot read IO tensors, so bounce 64B through DRAM.)
        wz = const.tile([1, 16], f32)
        nc.vector.memset(wz, 1.0)
        warm_src = dram.tile([1, 16], f32, name="warm_src", tag="warm_src")
        nc.gpsimd.dma_start(out=warm_src, in_=wz)
        warm = dram.tile([1, 16], f32, name="warm", tag="warm")
        nc.gpsimd.collective_compute(
            "AllReduce", ALU.add,
            replica_groups=[[0, 1, 2, 3], [4, 5, 6, 7]],
            ins=[warm_src.opt()], outs=[warm.opt()])

        # outputs of phase A live until end of attention (phase B) only
        estAB = contextlib.ExitStack()
        poolA_out = estAB.enter_context(tc.tile_pool(name="poolA_out", bufs=1))
        qkT = poolA_out.tile([P, 2, N], bf16)         # q^T, k^T feature-major
        v_sb = poolA_out.tile([P, NRT, H2 * DV], f8)  # v row-major [p,mt,c]

        # ---------------- Phase A: LN1 + transposes + qkv ----------------
        with (
            tc.tile_pool(name="poolA", bufs=1) as poolA,
            tc.tile_pool(name="streamA", bufs=3) as streamA,
            tc.tile_pool(name="psumA", bufs=2, space="PSUM") as psumA,
        ):
            xnT = poolA.tile([P, KC, N], bf16)  # feature-major normalized x

            # LN1 statistics batched per row-quad: one reciprocal per 4 rows
            # (DVE reciprocal has a ~0.6us fixed cost) keeps the PE fed
            for rq in range(NRT // 4):
                xts, mvs = [], []
                for j in range(4):
                    rt = rq * 4 + j
                    x_t = streamA.tile([P, D], f32, tag="x_t", bufs=6,
                                       name=f"x_t{rt}")
                    nc.sync.dma_start(out=x_t, in_=x[rt * P:(rt + 1) * P, :])
                    st6 = streamA.tile([P, 6], f32, tag="st6", bufs=4)
                    nc.vector.bn_stats(out=st6, in_=x_t)
                    mv = streamA.tile([P, 2], f32, tag="mv", bufs=6,
                                      name=f"mv{rt}")
                    nc.vector.bn_aggr(out=mv, in_=st6)
                    xts.append(x_t)
                    mvs.append(mv)
                sd4 = streamA.tile([P, 4], f32, tag="sd4")
                for j in range(4):
                    nc.scalar.activation(out=sd4[:, j:j + 1],
                                         in_=mvs[j][:, 1:2], func=AF.Sqrt,
                                         bias=eps_t, scale=1.0)
                rstd4 = streamA.tile([P, 4], f32, tag="rstd4")
                nc.vector.reciprocal(out=rstd4, in_=sd4)
                for j in range(4):
                    rt = rq * 4 + j
                    xn_t = streamA.tile([P, D], bf16, tag="xn_t")
                    nc.vector.tensor_scalar(out=xn_t, in0=xts[j],
                                            scalar1=mvs[j][:, 0:1],
                                            scalar2=rstd4[:, j:j + 1],
                                            op0=ALU.subtract, op1=ALU.mult)
                    for kc in range(KC):
                        psT = psumA.tile([P, P], bf16, tag="psT")
                        nc.tensor.transpose(psT, xn_t[:, kc * P:(kc + 1) * P],
                                            ident)
                        nc.scalar.copy(out=xnT[:, kc, rt * P:(rt + 1) * P],
                                       in_=psT)

            # q^T / k^T: feature-major [col, rows]
            for ct in range(2):
                for rr in range(4):
                    ps = psumA.tile([P, QB], f32, tag="ps_qk")
                    for kc in range(KC):
                        nc.tensor.matmul(
                            ps, wqkv_sb[:, kc, ct * P:(ct + 1) * P],
                            xnT[:, kc, rr * QB:(rr + 1) * QB],
                            start=(kc == 0), stop=(kc == KC - 1))
                    nc.scalar.activation(
                        out=qkT[:, ct, rr * QB:(rr + 1) * QB], in_=ps,
                        func=AF.Identity, bias=bqk_pt[:, ct:ct + 1], scale=1.0)

            # v: row-major [m, c] (c = 2 heads x 512)
            for mt in range(NRT):
                for cr in range(2):
                    ps = psumA.tile([P, DV], f32, tag="ps_v")
                    for kc in range(KC):
                        nc.tensor.matmul(
                            ps, xnT[:, kc, mt * P:(mt + 1) * P],
                            wqkv_sb[:, kc, 2 * H2 * DH + cr * DV:
                                    2 * H2 * DH + (cr + 1) * DV],
                            start=(kc == 0), stop=(kc == KC - 1))
                    nc.vector.tensor_tensor(
                        out=v_sb[:, mt, cr * DV:(cr + 1) * DV], in0=ps,
                        in1=bv_b[:, cr, :], op=ALU.add)

        # ------------- Phases B + C interleaved per q-block -------------
        with (
            tc.tile_pool(name="poolC", bufs=1) as poolC,
            tc.tile_pool(name="streamB", bufs=2) as streamB,
            tc.tile_pool(name="streamC", bufs=2) as streamC,
            tc.tile_pool(name="psumBC", bufs=2, space="PSUM") as psum,
        ):
            x2_sb = poolC.tile([P, NQB, D], f32)
            xn2T = poolC.tile([P, KC, NQB * P], bf16)
            hT = poolC.tile([P, EC, NQB * P], bf16)

            def phaseB(qb):
                oT = streamB.tile([P, H2 * DV // P, QB], bf16, tag="oT")
                rd = [None, None]
                # scores for BOTH heads first, at full matmul rate: a DVE
                # copy frees each ps_s slot immediately instead of at exp's
                # ~660ns/tile pace; exp reads the SBUF stage at its own pace
                # and its latency hides under the first head's attn@v
                eTs = []
                for hh in range(H2):
                    hp = slice(DH * hh, DH * (hh + 1))
                    eT = streamB.tile([P, NRT, QB], f8, tag="eT",
                                      name=f"eT{hh}")
                    eTs.append(eT)
                    for kt in range(NRT):
                        ps_s = psum.tile([P, QB], f32, tag="ps_s", bufs=3)
                        nc.tensor.matmul(
                            ps_s, qkT[hp, 1, kt * P:(kt + 1) * P],
                            qkT[hp, 0, qb * QB:(qb + 1) * QB],
                            start=True, stop=True)
                        sb_sc = streamB.tile([P, QB], f32, tag="sb_sc",
                                             bufs=8, name="sb_sc")
                        nc.vector.tensor_copy(out=sb_sc, in_=ps_s)
                        nc.scalar.activation(out=eT[:, kt, :], in_=sb_sc,
                                             func=AF.Exp, scale=SCALE,
                                             bias=ln4_t)
                for hh in range(H2):
                    eT = eTs[hh]
                    # denominator rows via fp8 DoubleRow ones-matmul (16
                    # identical rows; dual-fp8 LDW needs M>=16, 16B steps)
                    ps_d = psum.tile([16, QB], f32, tag="ps_d", bufs=1)
                    for kt in range(0, NRT, 2):
                        nc.tensor.matmul(ps_d, ones2_f8, eT[:, kt:kt + 2, :],
                                         start=(kt == 0), stop=(kt == NRT - 2),
                                         perf_mode=DR)
                    d_sb = streamB.tile([1, QB], bf16, tag="d_sb")
                    nc.vector.tensor_copy(out=d_sb, in_=ps_d[0:1, :])
                    # transpose to [q-partition, qt] layout, then wide recip
                    rd_raw = streamB.tile([P, QB // P], f32, tag="rd_raw")
                    for qt in range(QB // P):
                        psd_t = psum.tile([P, 1], bf16, tag="ps_av")
                        nc.tensor.transpose(
                            psd_t, d_sb[0:1, qt * P:(qt + 1) * P],
                            ident[0:1, 0:1])
                        nc.vector.tensor_copy(out=rd_raw[:, qt:qt + 1],
                                              in_=psd_t)
                    rd[hh] = streamB.tile([P, QB // P], f32, tag="rd",
                                          name=f"rd{hh}")
                    nc.vector.reciprocal(out=rd[hh], in_=rd_raw)
                    for ct in range(DV // P):
                        ps_av = psum.tile([P, QB], f32, tag="ps_av")
                        for mc in range(0, NRT, 2):
                            nc.tensor.matmul(
                                ps_av,
                                v_sb[:, mc:mc + 2,
                                     hh * DV + ct * P:hh * DV + (ct + 1) * P],
                                eT[:, mc:mc + 2, :],
                                start=(mc == 0), stop=(mc == NRT - 2),
                                perf_mode=DR)
                        nc.vector.tensor_copy(
                            out=oT[:, hh * (DV // P) + ct, :], in_=ps_av)

                # merge-proj partial, normalized per head by rd (per-partition
                # scalars), bm/4 folded in -> rs_in[qb]
                for qt in range(QB // P):
                    ps_m0 = psum.tile([P, D], f32, tag="ps_m")
                    for ch in range(4):
                        nc.tensor.matmul(
                            ps_m0, oT[:, ch, qt * P:(qt + 1) * P], wm_sb[:, ch, :],
                            start=(ch == 0), stop=(ch == 3))
                    pt0 = streamB.tile([P, D], f32, tag="pt0")
                    nc.vector.scalar_tensor_tensor(
                        out=pt0, in0=ps_m0, scalar=rd[0][:, qt:qt + 1],
                        in1=bm4_b, op0=ALU.mult, op1=ALU.add)
                    ps_m1 = psum.tile([P, D], f32, tag="ps_m")
                    for ch in range(4, 8):
                        nc.tensor.matmul(
                            ps_m1, oT[:, ch, qt * P:(qt + 1) * P], wm_sb[:, ch, :],
                            start=(ch == 4), stop=(ch == 7))
                    pt_sb = streamB.tile([P, D], bf16, tag="pt_sb", bufs=3)
                    nc.vector.scalar_tensor_tensor(
                        out=pt_sb, in0=ps_m1, scalar=rd[1][:, qt:qt + 1],
                        in1=pt0, op0=ALU.mult, op1=ALU.add)
                    nc.sync.dma_start(out=rs_in[qb][qt * P:(qt + 1) * P, :],
                                      in_=pt_sb)

                nc.gpsimd.collective_compute(
                    "ReduceScatter", ALU.add,
                    replica_groups=[[0, 1, 2, 3], [4, 5, 6, 7]],
                    ins=[rs_in[qb].opt()], outs=[rs_out[qb].opt()])

            def phaseC_x2(qb):
                rs_t = streamC.tile([P, D], bf16, tag="rs_t")
                nc.sync.dma_start(out=rs_t, in_=rs_out[qb][:, :])
                rs_f = streamC.tile([P, D], f32, tag="rs_f")
                nc.vector.tensor_copy(out=rs_f, in_=rs_t)
                nc.vector.tensor_tensor(out=x2_sb[:, qb, :], in0=rs_f,
                                        in1=xr_sb[:, qb, :], op=ALU.add)
                st6 = streamC.tile([P, 6], f32, tag="st6c")
                nc.vector.bn_stats(out=st6, in_=x2_sb[:, qb, :])
                mv = streamC.tile([P, 2], f32, tag="mvc")
                nc.vector.bn_aggr(out=mv, in_=st6)
                sd = streamC.tile([P, 1], f32, tag="sdc")
                nc.scalar.activation(out=sd, in_=mv[:, 1:2], func=AF.Sqrt,
                                     bias=eps_t, scale=1.0)
                rstd = streamC.tile([P, 1], f32, tag="rstdc")
                nc.vector.reciprocal(out=rstd, in_=sd)
                xn2_t = streamC.tile([P, D], bf16, tag="xn2_t")
                nc.vector.tensor_scalar(out=xn2_t, in0=x2_sb[:, qb, :],
                                        scalar1=mv[:, 0:1], scalar2=rstd,
                                        op0=ALU.subtract, op1=ALU.mult)
                for kc in range(KC):
                    psT = psum.tile([P, P], bf16, tag="ps_s", bufs=3)
                    nc.tensor.transpose(psT, xn2_t[:, kc * P:(kc + 1) * P], ident)
                    nc.vector.tensor_copy(out=xn2T[:, kc, qb * P:(qb + 1) * P],
                                          in_=psT)

            def ff1_part(c0, w):
                cols = slice(c0, c0 + w)  # q columns
                for et in range(EC):
                    ps_h = psum.tile([P, w], f32, tag="ps_av", name="ps_h")
                    for kc in range(KC):
                        nc.tensor.matmul(ps_h, w1_sb[:, kc, et * P:(et + 1) * P],
                                         xn2T[:, kc, cols],
                                         start=(kc == 0), stop=(kc == KC - 1))
                    nc.scalar.activation(out=hT[:, et, cols], in_=ps_h,
                                         func=AF.Silu,
                                         bias=b1_pt[:, et:et + 1], scale=1.0)

            def ff2(qt):
                ps_o = psum.tile([P, D], f32, tag="ps_m")
                for ec in range(EC):
                    nc.tensor.matmul(ps_o, hT[:, ec, qt * P:(qt + 1) * P],
                                     w2_sb[:, ec, :],
                                     start=(ec == 0), stop=(ec == EC - 1))
                o_t = streamC.tile([P, D], f32, tag="o_t")
                nc.vector.tensor_tensor(out=o_t, in0=ps_o, in1=x2_sb[:, qt, :],
                                        op=ALU.add)
                nc.vector.tensor_tensor(out=o_t, in0=o_t, in1=b2_b, op=ALU.add)
                nc.sync.dma_start(out=out[qt * P:(qt + 1) * P, :], in_=o_t)

            # schedule: keep PE fed while collectives land (RS rendezvous can
            # lag ~10-25us behind the local merge, so consume each rs_out two
            # q-blocks later)
            phaseB(0)
            phaseB(1)
            phaseB(2)
            phaseC_x2(0)
            phaseB(3)
            phaseC_x2(1)
            ff1_part(0, 2 * P)      # qb0+qb1 columns
            phaseC_x2(2)
            ff2(0)
            ff2(1)
            phaseC_x2(3)
            ff1_part(2 * P, 2 * P)  # qb2+qb3 columns
            ff2(2)
            ff2(3)

        estAB.close()


def build_nc():
    nc = bacc.Bacc("TRN2", target_bir_lowering=False, debug=False, num_devices=8)
    x = nc.dram_tensor("x", [N, D], f32, kind="ExternalInput")
    xr = nc.dram_tensor("xr", [NQB, P, D], f32, kind="ExternalInput")
    wqkv = nc.dram_tensor("wqkv", [D, 2 * H2 * DH + H2 * DV], bf16,
                          kind="ExternalInput")
    bqk_pt = nc.dram_tensor("bqk_pt", [P, 2], f32, kind="ExternalInput")
    bv = nc.dram_tensor("bv", [1, H2 * DV], f32, kind="ExternalInput")
    wm = nc.dram_tensor("wm", [H2 * DV, D], bf16, kind="ExternalInput")
    bm4 = nc.dram_tensor("bm4", [1, D], f32, kind="ExternalInput")
    w1 = nc.dram_tensor("w1", [D, E], bf16, kind="ExternalInput")
    b1_pt = nc.dram_tensor("b1_pt", [P, EC], f32, kind="ExternalInput")
    w2 = nc.dram_tensor("w2", [E, D], bf16, kind="ExternalInput")
    b2 = nc.dram_tensor("b2", [1, D], f32, kind="ExternalInput")

    outs = {"out": nc.dram_tensor("out", [NQB * P, D], f32,
                                  kind="ExternalOutput").ap()}
    ins = (x.ap(), xr.ap(), wqkv.ap(), bqk_pt.ap(), bv.ap(), wm.ap(),
           bm4.ap(), w1.ap(), b1_pt.ap(), w2.ap(), b2.ap())
    with tile.TileContext(nc) as tc:
        build_body(tc, ins, outs)
    nc.compile()
    return nc


def make_in_maps(inputs):
    """inputs: dict from reference.setup_inputs() (numpy f32). Returns list of 8 in_maps."""
    bf = ml_dtypes.bfloat16
    x = np.asarray(inputs["x"], np.float32)
    ln1_g = np.asarray(inputs["ln1_g"], np.float32)
    ln1_b = np.asarray(inputs["ln1_b"], np.float32)
    Wqkv = np.asarray(inputs["Wqkv"], np.float32)
    bqkv = np.asarray(inputs["bqkv"], np.float32)
    Wm = np.asarray(inputs["Wm"], np.float32)
    bm = np.asarray(inputs["bm"], np.float32)
    ln2_g = np.asarray(inputs["ln2_g"], np.float32)
    ln2_b = np.asarray(inputs["ln2_b"], np.float32)
    W1 = np.asarray(inputs["W1"], np.float32)
    b1 = np.asarray(inputs["b1"], np.float32)
    W2 = np.asarray(inputs["W2"], np.float32)
    b2 = np.asarray(inputs["b2"], np.float32)

    Wqkv_eff = ln1_g[:, None] * Wqkv
    bqkv_eff = ln1_b @ Wqkv + bqkv
    W1_eff = ln2_g[:, None] * W1
    b1_eff = ln2_b @ W1 + b1

    DQ = 512
    in_maps = []
    for c in range(8):
        b = c // 4
        g = c % 4
        qcols = slice(DH * 2 * g, DH * 2 * g + 2 * DH)
        kcols = slice(DQ + DH * 2 * g, DQ + DH * 2 * g + 2 * DH)
        vcols = slice(2 * DQ + H2 * DV * g, 2 * DQ + H2 * DV * (g + 1))
        wqkv_c = np.concatenate(
            [Wqkv_eff[:, qcols], Wqkv_eff[:, kcols], Wqkv_eff[:, vcols]], axis=1)
        bq = bqkv_eff[qcols]
        bk = bqkv_eff[kcols]
        bv_c = bqkv_eff[vcols]
        bqk_pt = np.stack([bq, bk], axis=1)  # [128, 2]
        wm_c = Wm[H2 * DV * g:H2 * DV * (g + 1), :]
        rank = g
        xr = np.stack([x[b, QB * j + P * rank:QB * j + P * (rank + 1), :]
                       for j in range(NQB)])
        in_maps.append({
            "x": np.ascontiguousarray(x[b]),
            "xr": np.ascontiguousarray(xr),
            "wqkv": np.ascontiguousarray(wqkv_c.astype(bf)),
            "bqk_pt": np.ascontiguousarray(bqk_pt),
            "bv": np.ascontiguousarray(bv_c[None, :]),
            "wm": np.ascontiguousarray(wm_c.astype(bf)),
            "bm4": np.ascontiguousarray((bm / 4.0)[None, :].astype(np.float32)),
            "w1": np.ascontiguousarray(W1_eff.astype(bf)),
            "b1_pt": np.ascontiguousarray(b1_eff.reshape(EC, P).T),
            "w2": np.ascontiguousarray(W2.astype(bf)),
            "b2": np.ascontiguousarray(b2[None, :]),
        })
    return in_maps


def assemble_output(results):
    """results: list of 8 dicts with 'out' [512, 512]. Returns (2, 2048, 512)."""
    full = np.empty((2, N, D), np.float32)
    for c in range(8):
        b, rank = c // 4, c % 4
        o = results[c]["out"]
        for j in range(NQB):
            full[b, QB * j + P * rank:QB * j + P * (rank + 1), :] = \
                o[P * j:P * (j + 1), :]
    return full


_NC_CACHE = {}


def kernel(**inputs) -> np.ndarray:
    """Full-input entry point: shards across 8 NeuronCores, returns full output."""
    key = "nc8"
    if key not in _NC_CACHE:
        _NC_CACHE[key] = build_nc()
    nc = _NC_CACHE[key]
    in_maps = make_in_maps(inputs)
    res = bass_utils.run_bass_kernel_spmd(nc, in_maps, core_ids=list(range(8)))
    return assemble_output(res.results)


# revision 37
# speedup vs baseline: 1.0637x; 1.0637x over previous
"""AttentionBlock Trainium2 kernel — 8-core SPMD, bf16 matmul path.

Sharding: core c -> batch b=c//4, head-pair g=c%4 (heads 2g, 2g+1).
Per core: LN1(all 2048 rows of batch b) -> qkv proj for its 2 heads ->
attention -> per-head merge-proj partials (bm/4 folded in) ->
ReduceScatter(+, bf16) within the 4-core batch group per 512-row
q-block -> each core owns 4x128 rows of x2 -> LN2 + FF(Swish) +
residual -> output chunk [512, 512] f32.

v2 changes vs v1:
- all matmuls bf16 (FWL halves weight-load overhead; psum stays f32)
- softmax denominator via ones[128,128] stationary matmul accumulation
  -> full-width [128,512] reciprocal (was [1,512] single-lane, 6us)
- weights prefetched on the gpsimd DMA queue (x stream on sync queue)
- ReduceScatter payload bf16 (half the collective time)
- phase C emitted per-qb interleaved with phase B; FF1 split in halves
  so only the last quarter of FF work sits behind the final RS
"""

import numpy as np
import ml_dtypes
import concourse.bass as bass
import concourse.bacc as bacc
import concourse.mybir as mybir
import concourse.tile as tile
from concourse import bass_utils
from concourse.masks import make_identity

P = 128
N = 2048          # sequence length
D = 512           # d_in / d_out
H2 = 2            # heads per core
DH = 64           # head dim (q, k)
DV = 512          # per-head value dim
E = 2048          # ff expand
QB = 512          # query block
NQB = N // QB     # 4
NRT = N // P      # 16 row tiles
KC = D // P       # 4 contraction chunks of d_in
EC = E // P       # 16 contraction chunks of d_expand
EPS = 1e-5
SCALE = DH ** -0.5

f32 = mybir.dt.float32
bf16 = mybir.dt.bfloat16
f8 = mybir.dt.float8e4
DR = mybir.MatmulPerfMode.DoubleRow
LN16 = 2.772588722239781  # exp(s - ln16): e4m3 saturates at 448, max s ~7.6

AF = mybir.ActivationFunctionType
ALU = mybir.AluOpType


def bcast_ap(ap, parts, free):
    """Partition-broadcast read AP for a [1, free] DRAM tensor."""
    return bass.AP(tensor=ap.tensor, offset=ap.offset, ap=[[0, parts], [1, free]])


def build_body(tc, ins, outs):
    nc = tc.nc
    (x, xr, wqkv, bqk_pt_d, bv_d, wm, bm4_d, w1, b1_pt_d, w2, b2_d) = ins
    out = outs["out"]

    import contextlib
    est = contextlib.ExitStack()
    with est:
        const = est.enter_context(tc.tile_pool(name="const", bufs=1))
        dram = est.enter_context(tc.tile_pool(name="dram", bufs=1, space="DRAM"))

        ident_f = const.tile([P, P], f32)
        make_identity(nc, ident_f)
        ident = const.tile([P, P], bf16)
        nc.vector.tensor_copy(ident, ident_f)
        ones2_f8 = const.tile([P, 2, 16], f8)
        nc.vector.memset(ones2_f8, 1.0)
        eps_t = const.tile([P, 1], f32)
        nc.vector.memset(eps_t, EPS)
        ln4_t = const.tile([P, 1], f32)
        nc.vector.memset(ln4_t, -LN16)

        # small constants + residual rows on the gpsimd DMA queue
        bqk_pt = const.tile([P, 2], f32)
        nc.gpsimd.dma_start(out=bqk_pt, in_=bqk_pt_d[:, :])
        bv_b = const.tile([P, 2, DV], f32)
        nc.gpsimd.dma_start(out=bv_b, in_=bcast_ap(bv_d, P, 2 * DV))
        bm4_b = const.tile([P, D], f32)
        nc.gpsimd.dma_start(out=bm4_b, in_=bcast_ap(bm4_d, P, D))
        b1_pt = const.tile([P, EC], f32)
        nc.gpsimd.dma_start(out=b1_pt, in_=b1_pt_d[:, :])
        b2_b = const.tile([P, D], f32)
        nc.gpsimd.dma_start(out=b2_b, in_=bcast_ap(b2_d, P, D))

        # persistent weights (prefetched early, gpsimd queue)
        poolW = est.enter_context(tc.tile_pool(name="poolW", bufs=1))
        wqkv_sb = poolW.tile([P, KC, 2 * H2 * DH + H2 * DV], bf16)
        wqkv_r = wqkv.rearrange("(c p) n -> p c n", p=P)
        for kc in range(KC):
            nc.gpsimd.dma_start(out=wqkv_sb[:, kc, :], in_=wqkv_r[:, kc, :])
        wm_sb = poolW.tile([P, H2 * DV // P, D], bf16)
        nc.gpsimd.dma_start(out=wm_sb, in_=wm.rearrange("(c p) n -> p c n", p=P))
        xr_sb = poolW.tile([P, NQB, D], f32)
        nc.gpsimd.dma_start(out=xr_sb, in_=xr.rearrange("q p d -> p q d"))
        w1_sb = poolW.tile([P, KC, E], bf16)
        w1r = w1.rearrange("(c p) n -> p c n", p=P)
        for kc in range(KC):
            nc.gpsimd.dma_start(out=w1_sb[:, kc, :], in_=w1r[:, kc, :])
        w2_sb = poolW.tile([P, EC, D], bf16)
        w2r = w2.rearrange("(c p) n -> p c n", p=P)
        for j in range(4):
            nc.gpsimd.dma_start(out=w2_sb[:, 4 * j:4 * (j + 1), :],
                                in_=w2r[:, 4 * j:4 * (j + 1), :])

        # DRAM bounce buffers for the ReduceScatter (bf16, one per q-block)
        rs_in = [dram.tile([QB, D], bf16, name=f"rs_in{j}", tag=f"rs_in{j}")
                 for j in range(NQB)]
        rs_out = [dram.tile([P, D], bf16, name=f"rs_out{j}", tag=f"rs_out{j}")
                  for j in range(NQB)]

        # Dummy tiny collective, first in the CC pipeline: absorbs cross-core
        # launch skew (~30us peer-wait otherwise paid by RS(0), delaying the
        # whole serialized collective spine) while the PE runs phase A.
        # (Collectives cannot read IO tensors, so bounce 64B through DRAM.)
        wz = const.tile([1, 16], f32)
        nc.vector.memset(wz, 1.0)
        warm_src = dram.tile([1, 16], f32, name="warm_src", tag="warm_src")
        nc.gpsimd.dma_start(out=warm_src, in_=wz)
        warm = dram.tile([1, 16], f32, name="warm", tag="warm")
        nc.gpsimd.collective_compute(
            "AllReduce", ALU.add,
            replica_groups=[[0, 1, 2, 3], [4, 5, 6, 7]],
            ins=[warm_src.opt()], outs=[warm.opt()])

        # outputs of phase A live until end of attention (phase B) only
        estAB = contextlib.ExitStack()
        poolA_out = estAB.enter_context(tc.tile_pool(name="poolA_out", bufs=1))
        qkT = poolA_out.tile([P, 2, N], bf16)         # q^T, k^T feature-major
        v_sb = poolA_out.tile([P, NRT, H2 * DV], f8)  # v row-major [p,mt,c]

        # ---------------- Phase A: LN1 + transposes + qkv ----------------
        with (
            tc.tile_pool(name="poolA", bufs=1) as poolA,
            tc.tile_pool(name="streamA", bufs=3) as streamA,
            tc.tile_pool(name="psumA", bufs=2, space="PSUM") as psumA,
        ):
            xnT = poolA.tile([P, KC, N], bf16)  # feature-major normalized x

            # LN1 statistics batched per row-quad: one reciprocal per 4 rows
            # (DVE reciprocal has a ~0.6us fixed cost) keeps the PE fed
            for rq in range(NRT // 4):
                xts, mvs = [], []
                for j in range(4):
                    rt = rq * 4 + j
                    x_t = streamA.tile([P, D], f32, tag="x_t", bufs=6,
                                       name=f"x_t{rt}")
                    nc.sync.dma_start(out=x_t, in_=x[rt * P:(rt + 1) * P, :])
                    st6 = streamA.tile([P, 6], f32, tag="st6", bufs=4)
                    nc.vector.bn_stats(out=st6, in_=x_t)
                    mv = streamA.tile([P, 2], f32, tag="mv", bufs=6,
                                      name=f"mv{rt}")
                    nc.vector.bn_aggr(out=mv, in_=st6)
                    xts.append(x_t)
                    mvs.append(mv)
                sd4 = streamA.tile([P, 4], f32, tag="sd4")
                for j in range(4):
                    nc.scalar.activation(out=sd4[:, j:j + 1],
                                         in_=mvs[j][:, 1:2], func=AF.Sqrt,
                                         bias=eps_t, scale=1.0)
                rstd4 = streamA.tile([P, 4], f32, tag="rstd4")
                nc.vector.reciprocal(out=rstd4, in_=sd4)
                for j in range(4):
                    rt = rq * 4 + j
                    xn_t = streamA.tile([P, D], bf16, tag="xn_t")
                    nc.vector.tensor_scalar(out=xn_t, in0=xts[j],
                                            scalar1=mvs[j][:, 0:1],
                                            scalar2=rstd4[:, j:j + 1],
                                            op0=ALU.subtract, op1=ALU.mult)
                    for kc in range(KC):
                        psT = psumA.tile([P, P], bf16, tag="psT")
                        nc.tensor.transpose(psT, xn_t[:, kc * P:(kc + 1) * P],
                                            ident)
                        nc.scalar.copy(out=xnT[:, kc, rt * P:(rt + 1) * P],
                                       in_=psT)

            # q^T / k^T: feature-major [col, rows]
            for ct in range(2):
                for rr in range(4):
                    ps = psumA.tile([P, QB], f32, tag="ps_qk")
                    for kc in range(KC):
                        nc.tensor.matmul(
                            ps, wqkv_sb[:, kc, ct * P:(ct + 1) * P],
                            xnT[:, kc, rr * QB:(rr + 1) * QB],
                            start=(kc == 0), stop=(kc == KC - 1))
                    nc.scalar.activation(
                        out=qkT[:, ct, rr * QB:(rr + 1) * QB], in_=ps,
                        func=AF.Identity, bias=bqk_pt[:, ct:ct + 1], scale=1.0)

            # v: row-major [m, c] (c = 2 heads x 512)
            for mt in range(NRT):
                for cr in range(2):
                    ps = psumA.tile([P, DV], f32, tag="ps_v")
                    for kc in range(KC):
                        nc.tensor.matmul(
                            ps, xnT[:, kc, mt * P:(mt + 1) * P],
                            wqkv_sb[:, kc, 2 * H2 * DH + cr * DV:
                                    2 * H2 * DH + (cr + 1) * DV],
                            start=(kc == 0), stop=(kc == KC - 1))
                    nc.vector.tensor_tensor(
                        out=v_sb[:, mt, cr * DV:(cr + 1) * DV], in0=ps,
                        in1=bv_b[:, cr, :], op=ALU.add)

        # ------------- Phases B + C interleaved per q-block -------------
        with (
            tc.tile_pool(name="poolC", bufs=1) as poolC,
            tc.tile_pool(name="streamB", bufs=2) as streamB,
            tc.tile_pool(name="streamC", bufs=2) as streamC,
            tc.tile_pool(name="psumBC", bufs=2, space="PSUM") as psum,
        ):
            x2_sb = poolC.tile([P, NQB, D], f32)
            xn2T = poolC.tile([P, KC, NQB * P], bf16)
            hT = poolC.tile([P, EC, NQB * P], bf16)

            def phaseB(qb):
                oT = streamB.tile([P, H2 * DV // P, QB], bf16, tag="oT")
                rd = [None, None]
                for hh in range(H2):
                    hp = slice(DH * hh, DH * (hh + 1))
                    eT = streamB.tile([P, NRT, QB], f8, tag="eT")
                    for kt in range(NRT):
                        ps_s = psum.tile([P, QB], f32, tag="ps_s", bufs=3)
                        nc.tensor.matmul(
                            ps_s, qkT[hp, 1, kt * P:(kt + 1) * P],
                            qkT[hp, 0, qb * QB:(qb + 1) * QB],
                            start=True, stop=True)
                        nc.scalar.activation(out=eT[:, kt, :], in_=ps_s,
                                             func=AF.Exp, scale=SCALE,
                                             bias=ln4_t)
                    # denominator rows via fp8 DoubleRow ones-matmul (16
                    # identical rows; dual-fp8 LDW needs M>=16, 16B steps)
                    ps_d = psum.tile([16, QB], f32, tag="ps_d", bufs=1)
                    for kt in range(0, NRT, 2):
                        nc.tensor.matmul(ps_d, ones2_f8, eT[:, kt:kt + 2, :],
                                         start=(kt == 0), stop=(kt == NRT - 2),
                                         perf_mode=DR)
                    d_sb = streamB.tile([1, QB], bf16, tag="d_sb")
                    nc.vector.tensor_copy(out=d_sb, in_=ps_d[0:1, :])
                    # transpose to [q-partition, qt] layout, then wide recip
                    rd_raw = streamB.tile([P, QB // P], f32, tag="rd_raw")
                    for qt in range(QB // P):
                        psd_t = psum.tile([P, 1], bf16, tag="ps_av")
                        nc.tensor.transpose(
                            psd_t, d_sb[0:1, qt * P:(qt + 1) * P],
                            ident[0:1, 0:1])
                        nc.vector.tensor_copy(out=rd_raw[:, qt:qt + 1],
                                              in_=psd_t)
                    rd[hh] = streamB.tile([P, QB // P], f32, tag="rd",
                                          name=f"rd{hh}")
                    nc.vector.reciprocal(out=rd[hh], in_=rd_raw)
                    for ct in range(DV // P):
                        ps_av = psum.tile([P, QB], f32, tag="ps_av")
                        for mc in range(0, NRT, 2):
                            nc.tensor.matmul(
                                ps_av,
                                v_sb[:, mc:mc + 2,
                                     hh * DV + ct * P:hh * DV + (ct + 1) * P],
                                eT[:, mc:mc + 2, :],
                                start=(mc == 0), stop=(mc == NRT - 2),
                                perf_mode=DR)
                        nc.vector.tensor_copy(
                            out=oT[:, hh * (DV // P) + ct, :], in_=ps_av)

                # merge-proj partial, normalized per head by rd (per-partition
                # scalars), bm/4 folded in -> rs_in[qb]
                for qt in range(QB // P):
                    ps_m0 = psum.tile([P, D], f32, tag="ps_m")
                    for ch in range(4):
                        nc.tensor.matmul(
                            ps_m0, oT[:, ch, qt * P:(qt + 1) * P], wm_sb[:, ch, :],
                            start=(ch == 0), stop=(ch == 3))
                    pt0 = streamB.tile([P, D], f32, tag="pt0")
                    nc.vector.scalar_tensor_tensor(
                        out=pt0, in0=ps_m0, scalar=rd[0][:, qt:qt + 1],
                        in1=bm4_b, op0=ALU.mult, op1=ALU.add)
                    ps_m1 = psum.tile([P, D], f32, tag="ps_m")
                    for ch in range(4, 8):
                        nc.tensor.matmul(
                            ps_m1, oT[:, ch, qt * P:(qt + 1) * P], wm_sb[:, ch, :],
                            start=(ch == 4), stop=(ch == 7))
                    pt_sb = streamB.tile([P, D], bf16, tag="pt_sb", bufs=3)
                    nc.vector.scalar_tensor_tensor(
                        out=pt_sb, in0=ps_m1, scalar=rd[1][:, qt:qt + 1],
                        in1=pt0, op0=ALU.mult, op1=ALU.add)
                    nc.sync.dma_start(out=rs_in[qb][qt * P:(qt + 1) * P, :],
                                      in_=pt_sb)

                nc.gpsimd.collective_compute(
                    "ReduceScatter", ALU.add,
                    replica_groups=[[0, 1, 2, 3], [4, 5, 6, 7]],
                    ins=[rs_in[qb].opt()], outs=[rs_out[qb].opt()])

            def phaseC_x2(qb):
                rs_t = streamC.tile([P, D], bf16, tag="rs_t")
                nc.sync.dma_start(out=rs_t, in_=rs_out[qb][:, :])
                rs_f = streamC.tile([P, D], f32, tag="rs_f")
                nc.vector.tensor_copy(out=rs_f, in_=rs_t)
                nc.vector.tensor_tensor(out=x2_sb[:, qb, :], in0=rs_f,
                                        in1=xr_sb[:, qb, :], op=ALU.add)
                st6 = streamC.tile([P, 6], f32, tag="st6c")
                nc.vector.bn_stats(out=st6, in_=x2_sb[:, qb, :])
                mv = streamC.tile([P, 2], f32, tag="mvc")
                nc.vector.bn_aggr(out=mv, in_=st6)
                sd = streamC.tile([P, 1], f32, tag="sdc")
                nc.scalar.activation(out=sd, in_=mv[:, 1:2], func=AF.Sqrt,
                                     bias=eps_t, scale=1.0)
                rstd = streamC.tile([P, 1], f32, tag="rstdc")
                nc.vector.reciprocal(out=rstd, in_=sd)
                xn2_t = streamC.tile([P, D], bf16, tag="xn2_t")
                nc.vector.tensor_scalar(out=xn2_t, in0=x2_sb[:, qb, :],
                                        scalar1=mv[:, 0:1], scalar2=rstd,
                                        op0=ALU.subtract, op1=ALU.mult)
                for kc in range(KC):
                    psT = psum.tile([P, P], bf16, tag="ps_s", bufs=3)
                    nc.tensor.transpose(psT, xn2_t[:, kc * P:(kc + 1) * P], ident)
                    nc.vector.tensor_copy(out=xn2T[:, kc, qb * P:(qb + 1) * P],
                                          in_=psT)

            def ff1_part(c0, w):
                cols = slice(c0, c0 + w)  # q columns
                for et in range(EC):
                    ps_h = psum.tile([P, w], f32, tag="ps_av", name="ps_h")
                    for kc in range(KC):
                        nc.tensor.matmul(ps_h, w1_sb[:, kc, et * P:(et + 1) * P],
                                         xn2T[:, kc, cols],
                                         start=(kc == 0), stop=(kc == KC - 1))
                    nc.scalar.activation(out=hT[:, et, cols], in_=ps_h,
                                         func=AF.Silu,
                                         bias=b1_pt[:, et:et + 1], scale=1.0)

            def ff2(qt):
                ps_o = psum.tile([P, D], f32, tag="ps_m")
                for ec in range(EC):
                    nc.tensor.matmul(ps_o, hT[:, ec, qt * P:(qt + 1) * P],
                                     w2_sb[:, ec, :],
                                     start=(ec == 0), stop=(ec == EC - 1))
                o_t = streamC.tile([P, D], f32, tag="o_t")
                nc.vector.tensor_tensor(out=o_t, in0=ps_o, in1=x2_sb[:, qt, :],
                                        op=ALU.add)
                nc.vector.tensor_tensor(out=o_t, in0=o_t, in1=b2_b, op=ALU.add)
                nc.sync.dma_start(out=out[qt * P:(qt + 1) * P, :], in_=o_t)

            # schedule: keep PE fed while collectives land (RS rendezvous can
            # lag ~10-25us behind the local merge, so consume each rs_out two
            # q-blocks later)
            phaseB(0)
            phaseB(1)
            phaseB(2)
            phaseC_x2(0)
            phaseB(3)
            phaseC_x2(1)
            ff1_part(0, 2 * P)      # qb0+qb1 columns
            phaseC_x2(2)
            ff2(0)
            ff2(1)
            phaseC_x2(3)
            ff1_part(2 * P, 2 * P)  # qb2+qb3 columns
            ff2(2)
            ff2(3)

        estAB.close()


def build_nc():
    nc = bacc.Bacc("TRN2", target_bir_lowering=False, debug=False, num_devices=8)
    x = nc.dram_tensor("x", [N, D], f32, kind="ExternalInput")
    xr = nc.dram_tensor("xr", [NQB, P, D], f32, kind="ExternalInput")
    wqkv = nc.dram_tensor("wqkv", [D, 2 * H2 * DH + H2 * DV], bf16,
                          kind="ExternalInput")
    bqk_pt = nc.dram_tensor("bqk_pt", [P, 2], f32, kind="ExternalInput")
    bv = nc.dram_tensor("bv", [1, H2 * DV], f32, kind="ExternalInput")
    wm = nc.dram_tensor("wm", [H2 * DV, D], bf16, kind="ExternalInput")
    bm4 = nc.dram_tensor("bm4", [1, D], f32, kind="ExternalInput")
    w1 = nc.dram_tensor("w1", [D, E], bf16, kind="ExternalInput")
    b1_pt = nc.dram_tensor("b1_pt", [P, EC], f32, kind="ExternalInput")
    w2 = nc.dram_tensor("w2", [E, D], bf16, kind="ExternalInput")
    b2 = nc.dram_tensor("b2", [1, D], f32, kind="ExternalInput")

    outs = {"out": nc.dram_tensor("out", [NQB * P, D], f32,
                                  kind="ExternalOutput").ap()}
    ins = (x.ap(), xr.ap(), wqkv.ap(), bqk_pt.ap(), bv.ap(), wm.ap(),
           bm4.ap(), w1.ap(), b1_pt.ap(), w2.ap(), b2.ap())
    with tile.TileContext(nc) as tc:
        build_body(tc, ins, outs)
    nc.compile()
    return nc


def make_in_maps(inputs):
    """inputs: dict from reference.setup_inputs() (numpy f32). Returns list of 8 in_maps."""
    bf = ml_dtypes.bfloat16
    x = np.asarray(inputs["x"], np.float32)
    ln1_g = np.asarray(inputs["ln1_g"], np.float32)
    ln1_b = np.asarray(inputs["ln1_b"], np.float32)
    Wqkv = np.asarray(inputs["Wqkv"], np.float32)
    bqkv = np.asarray(inputs["bqkv"], np.float32)
    Wm = np.asarray(inputs["Wm"], np.float32)
    bm = np.asarray(inputs["bm"], np.float32)
    ln2_g = np.asarray(inputs["ln2_g"], np.float32)
    ln2_b = np.asarray(inputs["ln2_b"], np.float32)
    W1 = np.asarray(inputs["W1"], np.float32)
    b1 = np.asarray(inputs["b1"], np.float32)
    W2 = np.asarray(inputs["W2"], np.float32)
    b2 = np.asarray(inputs["b2"], np.float32)

    Wqkv_eff = ln1_g[:, None] * Wqkv
    bqkv_eff = ln1_b @ Wqkv + bqkv
    W1_eff = ln2_g[:, None] * W1
    b1_eff = ln2_b @ W1 + b1

    DQ = 512
    in_maps = []
    for c in range(8):
        b = c // 4
        g = c % 4
        qcols = slice(DH * 2 * g, DH * 2 * g + 2 * DH)
        kcols = slice(DQ + DH * 2 * g, DQ + DH * 2 * g + 2 * DH)
        vcols = slice(2 * DQ + H2 * DV * g, 2 * DQ + H2 * DV * (g + 1))
        wqkv_c = np.concatenate(
            [Wqkv_eff[:, qcols], Wqkv_eff[:, kcols], Wqkv_eff[:, vcols]], axis=1)
        bq = bqkv_eff[qcols]
        bk = bqkv_eff[kcols]
        bv_c = bqkv_eff[vcols]
        bqk_pt = np.stack([bq, bk], axis=1)  # [128, 2]
        wm_c = Wm[H2 * DV * g:H2 * DV * (g + 1), :]
        rank = g
        xr = np.stack([x[b, QB * j + P * rank:QB * j + P * (rank + 1), :]
                       for j in range(NQB)])
        in_maps.append({
            "x": np.ascontiguousarray(x[b]),
            "xr": np.ascontiguousarray(xr),
            "wqkv": np.ascontiguousarray(wqkv_c.astype(bf)),
            "bqk_pt": np.ascontiguousarray(bqk_pt),
            "bv": np.ascontiguousarray(bv_c[None, :]),
            "wm": np.ascontiguousarray(wm_c.astype(bf)),
            "bm4": np.ascontiguousarray((bm / 4.0)[None, :].astype(np.float32)),
            "w1": np.ascontiguousarray(W1_eff.astype(bf)),
            "b1_pt": np.ascontiguousarray(b1_eff.reshape(EC, P).T),
            "w2": np.ascontiguousarray(W2.astype(bf)),
            "b2": np.ascontiguousarray(b2[None, :]),
        })
    return in_maps


def assemble_output(results):
    """results: list of 8 dicts with 'out' [512, 512]. Returns (2, 2048, 512)."""
    full = np.empty((2, N, D), np.float32)
    for c in range(8):
        b, rank = c // 4, c % 4
        o = results[c]["out"]
        for j in range(NQB):
            full[b, QB * j + P * rank:QB * j + P * (rank + 1), :] = \
                o[P * j:P * (j + 1), :]
    return full


_NC_CACHE = {}


def kernel(**inputs) -> np.ndarray:
    """Full-input entry point: shards across 8 NeuronCores, returns full output."""
    key = "nc8"
    if key not in _NC_CACHE:
        _NC_CACHE[key] = build_nc()
    nc = _NC_CACHE[key]
    in_maps = make_in_maps(inputs)
    res = bass_utils.run_bass_kernel_spmd(nc, in_maps, core_ids=list(range(8)))
    return assemble_output(res.results)


# revision 38
# speedup vs baseline: 1.0788x; 1.0142x over previous
"""AttentionBlock Trainium2 kernel — 8-core SPMD, bf16 matmul path.

Sharding: core c -> batch b=c//4, head-pair g=c%4 (heads 2g, 2g+1).
Per core: LN1(all 2048 rows of batch b) -> qkv proj for its 2 heads ->
attention -> per-head merge-proj partials (bm/4 folded in) ->
ReduceScatter(+, bf16) within the 4-core batch group per 512-row
q-block -> each core owns 4x128 rows of x2 -> LN2 + FF(Swish) +
residual -> output chunk [512, 512] f32.

v2 changes vs v1:
- all matmuls bf16 (FWL halves weight-load overhead; psum stays f32)
- softmax denominator via ones[128,128] stationary matmul accumulation
  -> full-width [128,512] reciprocal (was [1,512] single-lane, 6us)
- weights prefetched on the gpsimd DMA queue (x stream on sync queue)
- ReduceScatter payload bf16 (half the collective time)
- phase C emitted per-qb interleaved with phase B; FF1 split in halves
  so only the last quarter of FF work sits behind the final RS
"""

import numpy as np
import ml_dtypes
import concourse.bass as bass
import concourse.bacc as bacc
import concourse.mybir as mybir
import concourse.tile as tile
from concourse import bass_utils
from concourse.masks import make_identity

P = 128
N = 2048          # sequence length
D = 512           # d_in / d_out
H2 = 2            # heads per core
DH = 64           # head dim (q, k)
DV = 512          # per-head value dim
E = 2048          # ff expand
QB = 512          # query block
NQB = N // QB     # 4
NRT = N // P      # 16 row tiles
KC = D // P       # 4 contraction chunks of d_in
EC = E // P       # 16 contraction chunks of d_expand
EPS = 1e-5
SCALE = DH ** -0.5

f32 = mybir.dt.float32
bf16 = mybir.dt.bfloat16
f8 = mybir.dt.float8e4
DR = mybir.MatmulPerfMode.DoubleRow
LN16 = 2.772588722239781  # exp(s - ln16): e4m3 saturates at 448, max s ~7.6

AF = mybir.ActivationFunctionType
ALU = mybir.AluOpType


def bcast_ap(ap, parts, free):
    """Partition-broadcast read AP for a [1, free] DRAM tensor."""
    return bass.AP(tensor=ap.tensor, offset=ap.offset, ap=[[0, parts], [1, free]])


def build_body(tc, ins, outs):
    nc = tc.nc
    (x, xr, wqkv, bqk_pt_d, bv_d, wm, bm4_d, w1, b1_pt_d, w2, b2_d) = ins
    out = outs["out"]

    import contextlib
    est = contextlib.ExitStack()
    with est:
        const = est.enter_context(tc.tile_pool(name="const", bufs=1))
        dram = est.enter_context(tc.tile_pool(name="dram", bufs=1, space="DRAM"))

        ident_f = const.tile([P, P], f32)
        make_identity(nc, ident_f)
        ident = const.tile([P, P], bf16)
        nc.vector.tensor_copy(ident, ident_f)
        ones2_f8 = const.tile([P, 2, 16], f8)
        nc.vector.memset(ones2_f8, 1.0)
        eps_t = const.tile([P, 1], f32)
        nc.vector.memset(eps_t, EPS)
        ln4_t = const.tile([P, 1], f32)
        nc.vector.memset(ln4_t, -LN16)

        # small constants + residual rows on the gpsimd DMA queue
        bqk_pt = const.tile([P, 2], f32)
        nc.gpsimd.dma_start(out=bqk_pt, in_=bqk_pt_d[:, :])
        bv_b = const.tile([P, 2, DV], f32)
        nc.gpsimd.dma_start(out=bv_b, in_=bcast_ap(bv_d, P, 2 * DV))
        bm4_b = const.tile([P, D], f32)
        nc.gpsimd.dma_start(out=bm4_b, in_=bcast_ap(bm4_d, P, D))
        b1_pt = const.tile([P, EC], f32)
        nc.gpsimd.dma_start(out=b1_pt, in_=b1_pt_d[:, :])
        b2_b = const.tile([P, D], f32)
        nc.gpsimd.dma_start(out=b2_b, in_=bcast_ap(b2_d, P, D))

        # persistent weights (prefetched early, gpsimd queue)
        poolW = est.enter_context(tc.tile_pool(name="poolW", bufs=1))
        wqkv_sb = poolW.tile([P, KC, 2 * H2 * DH + H2 * DV], bf16)
        wqkv_r = wqkv.rearrange("(c p) n -> p c n", p=P)
        for kc in range(KC):
            nc.gpsimd.dma_start(out=wqkv_sb[:, kc, :], in_=wqkv_r[:, kc, :])
        wm_sb = poolW.tile([P, H2 * DV // P, D], bf16)
        nc.gpsimd.dma_start(out=wm_sb, in_=wm.rearrange("(c p) n -> p c n", p=P))
        xr_sb = poolW.tile([P, NQB, D], f32)
        nc.gpsimd.dma_start(out=xr_sb, in_=xr.rearrange("q p d -> p q d"))
        w1_sb = poolW.tile([P, KC, E], bf16)
        w1r = w1.rearrange("(c p) n -> p c n", p=P)
        for kc in range(KC):
            nc.gpsimd.dma_start(out=w1_sb[:, kc, :], in_=w1r[:, kc, :])
        w2_sb = poolW.tile([P, EC, D], bf16)
        w2r = w2.rearrange("(c p) n -> p c n", p=P)
        for j in range(4):
            nc.gpsimd.dma_start(out=w2_sb[:, 4 * j:4 * (j + 1), :],
                                in_=w2r[:, 4 * j:4 * (j + 1), :])

        # DRAM bounce buffers for the ReduceScatter (bf16, one per q-block)
        rs_in = [dram.tile([QB, D], bf16, name=f"rs_in{j}", tag=f"rs_in{j}")
                 for j in range(NQB)]
        rs_out = [dram.tile([P, D], bf16, name=f"rs_out{j}", tag=f"rs_out{j}")
                  for j in range(NQB)]

        # Dummy tiny collective, first in the CC pipeline: absorbs cross-core
        # launch skew (~30us peer-wait otherwise paid by RS(0), delaying the
        # whole serialized collective spine) while the PE runs phase A.
        # (Collectives cannot read IO tensors, so bounce 64B through DRAM.)
        wz = const.tile([1, 16], f32)
        nc.vector.memset(wz, 1.0)
        warm_src = dram.tile([1, 16], f32, name="warm_src", tag="warm_src")
        nc.gpsimd.dma_start(out=warm_src, in_=wz)
        warm = dram.tile([1, 16], f32, name="warm", tag="warm")
        nc.gpsimd.collective_compute(
            "AllReduce", ALU.add,
            replica_groups=[[0, 1, 2, 3], [4, 5, 6, 7]],
            ins=[warm_src.opt()], outs=[warm.opt()])

        # outputs of phase A live until end of attention (phase B) only
        estAB = contextlib.ExitStack()
        poolA_out = estAB.enter_context(tc.tile_pool(name="poolA_out", bufs=1))
        qkT = poolA_out.tile([P, 2, N], bf16)         # q^T, k^T feature-major
        v_sb = poolA_out.tile([P, NRT, H2 * DV], f8)  # v row-major [p,mt,c]

        # ---------------- Phase A: LN1 + transposes + qkv ----------------
        with (
            tc.tile_pool(name="poolA", bufs=1) as poolA,
            tc.tile_pool(name="streamA", bufs=3) as streamA,
            tc.tile_pool(name="psumA", bufs=2, space="PSUM") as psumA,
        ):
            xnT = poolA.tile([P, KC, N], bf16)  # feature-major normalized x

            # LN1 statistics batched per row-quad: one reciprocal per 4 rows
            # (DVE reciprocal has a ~0.6us fixed cost) keeps the PE fed
            for rq in range(NRT // 4):
                xts, mvs = [], []
                for j in range(4):
                    rt = rq * 4 + j
                    x_t = streamA.tile([P, D], f32, tag="x_t", bufs=6,
                                       name=f"x_t{rt}")
                    nc.sync.dma_start(out=x_t, in_=x[rt * P:(rt + 1) * P, :])
                    st6 = streamA.tile([P, 6], f32, tag="st6", bufs=4)
                    nc.vector.bn_stats(out=st6, in_=x_t)
                    mv = streamA.tile([P, 2], f32, tag="mv", bufs=6,
                                      name=f"mv{rt}")
                    nc.vector.bn_aggr(out=mv, in_=st6)
                    xts.append(x_t)
                    mvs.append(mv)
                sd4 = streamA.tile([P, 4], f32, tag="sd4")
                for j in range(4):
                    nc.scalar.activation(out=sd4[:, j:j + 1],
                                         in_=mvs[j][:, 1:2], func=AF.Sqrt,
                                         bias=eps_t, scale=1.0)
                rstd4 = streamA.tile([P, 4], f32, tag="rstd4")
                nc.vector.reciprocal(out=rstd4, in_=sd4)
                for j in range(4):
                    rt = rq * 4 + j
                    xn_t = streamA.tile([P, D], bf16, tag="xn_t")
                    nc.vector.tensor_scalar(out=xn_t, in0=xts[j],
                                            scalar1=mvs[j][:, 0:1],
                                            scalar2=rstd4[:, j:j + 1],
                                            op0=ALU.subtract, op1=ALU.mult)
                    for kc in range(KC):
                        psT = psumA.tile([P, P], bf16, tag="psT")
                        nc.tensor.transpose(psT, xn_t[:, kc * P:(kc + 1) * P],
                                            ident)
                        nc.scalar.copy(out=xnT[:, kc, rt * P:(rt + 1) * P],
                                       in_=psT)

            # q^T / k^T: feature-major [col, rows]
            for ct in range(2):
                for rr in range(4):
                    ps = psumA.tile([P, QB], f32, tag="ps_qk")
                    for kc in range(KC):
                        nc.tensor.matmul(
                            ps, wqkv_sb[:, kc, ct * P:(ct + 1) * P],
                            xnT[:, kc, rr * QB:(rr + 1) * QB],
                            start=(kc == 0), stop=(kc == KC - 1))
                    nc.scalar.activation(
                        out=qkT[:, ct, rr * QB:(rr + 1) * QB], in_=ps,
                        func=AF.Identity, bias=bqk_pt[:, ct:ct + 1], scale=1.0)

            # v: row-major [m, c] (c = 2 heads x 512)
            for mt in range(NRT):
                for cr in range(2):
                    ps = psumA.tile([P, DV], f32, tag="ps_v")
                    for kc in range(KC):
                        nc.tensor.matmul(
                            ps, xnT[:, kc, mt * P:(mt + 1) * P],
                            wqkv_sb[:, kc, 2 * H2 * DH + cr * DV:
                                    2 * H2 * DH + (cr + 1) * DV],
                            start=(kc == 0), stop=(kc == KC - 1))
                    nc.vector.tensor_tensor(
                        out=v_sb[:, mt, cr * DV:(cr + 1) * DV], in0=ps,
                        in1=bv_b[:, cr, :], op=ALU.add)

        # ------------- Phases B + C interleaved per q-block -------------
        with (
            tc.tile_pool(name="poolC", bufs=1) as poolC,
            tc.tile_pool(name="streamB", bufs=2) as streamB,
            tc.tile_pool(name="streamC", bufs=2) as streamC,
            tc.tile_pool(name="psumBC", bufs=2, space="PSUM") as psum,
        ):
            x2_sb = poolC.tile([P, NQB, D], f32)
            xn2T = poolC.tile([P, KC, NQB * P], bf16)
            hT = poolC.tile([P, EC, NQB * P], bf16)

            def phaseB(qb):
                oT = streamB.tile([P, H2 * DV // P, QB], bf16, tag="oT")
                rd = [None, None]
                for hh in range(H2):
                    hp = slice(DH * hh, DH * (hh + 1))
                    eT = streamB.tile([P, NRT, QB], f8, tag="eT")
                    for kt in range(NRT):
                        ps_s = psum.tile([P, QB], f32, tag="ps_s", bufs=3)
                        nc.tensor.matmul(
                            ps_s, qkT[hp, 1, kt * P:(kt + 1) * P],
                            qkT[hp, 0, qb * QB:(qb + 1) * QB],
                            start=True, stop=True)
                        nc.scalar.activation(out=eT[:, kt, :], in_=ps_s,
                                             func=AF.Exp, scale=SCALE,
                                             bias=ln4_t)
                    # denominator rows via fp8 DoubleRow ones-matmul (16
                    # identical rows; dual-fp8 LDW needs M>=16, 16B steps)
                    ps_d = psum.tile([16, QB], f32, tag="ps_d", bufs=1)
                    for kt in range(0, NRT, 2):
                        nc.tensor.matmul(ps_d, ones2_f8, eT[:, kt:kt + 2, :],
                                         start=(kt == 0), stop=(kt == NRT - 2),
                                         perf_mode=DR)
                    d_sb = streamB.tile([1, QB], bf16, tag="d_sb")
                    nc.vector.tensor_copy(out=d_sb, in_=ps_d[0:1, :])
                    # transpose to [q-partition, qt] layout, then wide recip
                    rd_raw = streamB.tile([P, QB // P], f32, tag="rd_raw")
                    for qt in range(QB // P):
                        psd_t = psum.tile([P, 1], bf16, tag="ps_av")
                        nc.tensor.transpose(
                            psd_t, d_sb[0:1, qt * P:(qt + 1) * P],
                            ident[0:1, 0:1])
                        nc.vector.tensor_copy(out=rd_raw[:, qt:qt + 1],
                                              in_=psd_t)
                    rd[hh] = streamB.tile([P, QB // P], f32, tag="rd",
                                          name=f"rd{hh}")
                    nc.vector.reciprocal(out=rd[hh], in_=rd_raw)
                    for ct in range(DV // P):
                        ps_av = psum.tile([P, QB], f32, tag="ps_av")
                        for mc in range(0, NRT, 2):
                            nc.tensor.matmul(
                                ps_av,
                                v_sb[:, mc:mc + 2,
                                     hh * DV + ct * P:hh * DV + (ct + 1) * P],
                                eT[:, mc:mc + 2, :],
                                start=(mc == 0), stop=(mc == NRT - 2),
                                perf_mode=DR)
                        nc.vector.tensor_copy(
                            out=oT[:, hh * (DV // P) + ct, :], in_=ps_av)

                # merge-proj partial, normalized per head by rd (per-partition
                # scalars), bm/4 folded in -> rs_in[qb]
                for qt in range(QB // P):
                    ps_m0 = psum.tile([P, D], f32, tag="ps_m")
                    for ch in range(4):
                        nc.tensor.matmul(
                            ps_m0, oT[:, ch, qt * P:(qt + 1) * P], wm_sb[:, ch, :],
                            start=(ch == 0), stop=(ch == 3))
                    pt0 = streamB.tile([P, D], f32, tag="pt0")
                    nc.vector.scalar_tensor_tensor(
                        out=pt0, in0=ps_m0, scalar=rd[0][:, qt:qt + 1],
                        in1=bm4_b, op0=ALU.mult, op1=ALU.add)
                    ps_m1 = psum.tile([P, D], f32, tag="ps_m")
                    for ch in range(4, 8):
                        nc.tensor.matmul(
                            ps_m1, oT[:, ch, qt * P:(qt + 1) * P], wm_sb[:, ch, :],
                            start=(ch == 4), stop=(ch == 7))
                    pt_sb = streamB.tile([P, D], bf16, tag="pt_sb", bufs=3)
                    nc.vector.scalar_tensor_tensor(
                        out=pt_sb, in0=ps_m1, scalar=rd[1][:, qt:qt + 1],
                        in1=pt0, op0=ALU.mult, op1=ALU.add)
                    nc.sync.dma_start(out=rs_in[qb][qt * P:(qt + 1) * P, :],
                                      in_=pt_sb)

                nc.gpsimd.collective_compute(
                    "ReduceScatter", ALU.add,
                    replica_groups=[[0, 1, 2, 3], [4, 5, 6, 7]],
                    ins=[rs_in[qb].opt()], outs=[rs_out[qb].opt()])

            def phaseC_x2(qb):
                rs_t = streamC.tile([P, D], bf16, tag="rs_t")
                nc.sync.dma_start(out=rs_t, in_=rs_out[qb][:, :])
                rs_f = streamC.tile([P, D], f32, tag="rs_f")
                nc.vector.tensor_copy(out=rs_f, in_=rs_t)
                nc.vector.tensor_tensor(out=x2_sb[:, qb, :], in0=rs_f,
                                        in1=xr_sb[:, qb, :], op=ALU.add)
                st6 = streamC.tile([P, 6], f32, tag="st6c")
                nc.vector.bn_stats(out=st6, in_=x2_sb[:, qb, :])
                mv = streamC.tile([P, 2], f32, tag="mvc")
                nc.vector.bn_aggr(out=mv, in_=st6)
                sd = streamC.tile([P, 1], f32, tag="sdc")
                nc.scalar.activation(out=sd, in_=mv[:, 1:2], func=AF.Sqrt,
                                     bias=eps_t, scale=1.0)
                rstd = streamC.tile([P, 1], f32, tag="rstdc")
                nc.vector.reciprocal(out=rstd, in_=sd)
                xn2_t = streamC.tile([P, D], bf16, tag="xn2_t")
                nc.vector.tensor_scalar(out=xn2_t, in0=x2_sb[:, qb, :],
                                        scalar1=mv[:, 0:1], scalar2=rstd,
                                        op0=ALU.subtract, op1=ALU.mult)
                for kc in range(KC):
                    psT = psum.tile([P, P], bf16, tag="ps_s", bufs=3)
                    nc.tensor.transpose(psT, xn2_t[:, kc * P:(kc + 1) * P], ident)
                    nc.vector.tensor_copy(out=xn2T[:, kc, qb * P:(qb + 1) * P],
                                          in_=psT)

            def ff1_part(c0, w):
                cols = slice(c0, c0 + w)  # q columns
                for et in range(EC):
                    ps_h = psum.tile([P, w], f32, tag="ps_av", name="ps_h")
                    for kc in range(KC):
                        nc.tensor.matmul(ps_h, w1_sb[:, kc, et * P:(et + 1) * P],
                                         xn2T[:, kc, cols],
                                         start=(kc == 0), stop=(kc == KC - 1))
                    nc.scalar.activation(out=hT[:, et, cols], in_=ps_h,
                                         func=AF.Silu,
                                         bias=b1_pt[:, et:et + 1], scale=1.0)

            def ff2(qt):
                ps_o = psum.tile([P, D], f32, tag="ps_m")
                for ec in range(EC):
                    nc.tensor.matmul(ps_o, hT[:, ec, qt * P:(qt + 1) * P],
                                     w2_sb[:, ec, :],
                                     start=(ec == 0), stop=(ec == EC - 1))
                o_t = streamC.tile([P, D], f32, tag="o_t")
                nc.vector.tensor_tensor(out=o_t, in0=ps_o, in1=x2_sb[:, qt, :],
                                        op=ALU.add)
                nc.vector.tensor_tensor(out=o_t, in0=o_t, in1=b2_b, op=ALU.add)
                nc.sync.dma_start(out=out[qt * P:(qt + 1) * P, :], in_=o_t)

            # schedule: keep PE fed while collectives land (RS rendezvous can
            # lag ~10-25us behind the local merge, so consume each rs_out two
            # q-blocks later)
            # All of phase B precedes any RS consumption: in-order PE queues
            # mean a C-block waiting on a drift-late collective would stall
            # every instruction behind it, so give each rs_out maximal slack.
            phaseB(0)
            phaseB(1)
            phaseB(2)
            phaseB(3)
            phaseC_x2(0)
            phaseC_x2(1)
            ff1_part(0, 2 * P)      # qb0+qb1 columns
            phaseC_x2(2)
            ff2(0)
            ff2(1)
            phaseC_x2(3)
            ff1_part(2 * P, 2 * P)  # qb2+qb3 columns
            ff2(2)
            ff2(3)

        estAB.close()


def build_nc():
    nc = bacc.Bacc("TRN2", target_bir_lowering=False, debug=False, num_devices=8)
    x = nc.dram_tensor("x", [N, D], f32, kind="ExternalInput")
    xr = nc.dram_tensor("xr", [NQB, P, D], f32, kind="ExternalInput")
    wqkv = nc.dram_tensor("wqkv", [D, 2 * H2 * DH + H2 * DV], bf16,
                          kind="ExternalInput")
    bqk_pt = nc.dram_tensor("bqk_pt", [P, 2], f32, kind="ExternalInput")
    bv = nc.dram_tensor("bv", [1, H2 * DV], f32, kind="ExternalInput")
    wm = nc.dram_tensor("wm", [H2 * DV, D], bf16, kind="ExternalInput")
    bm4 = nc.dram_tensor("bm4", [1, D], f32, kind="ExternalInput")
    w1 = nc.dram_tensor("w1", [D, E], bf16, kind="ExternalInput")
    b1_pt = nc.dram_tensor("b1_pt", [P, EC], f32, kind="ExternalInput")
    w2 = nc.dram_tensor("w2", [E, D], bf16, kind="ExternalInput")
    b2 = nc.dram_tensor("b2", [1, D], f32, kind="ExternalInput")

    outs = {"out": nc.dram_tensor("out", [NQB * P, D], f32,
                                  kind="ExternalOutput").ap()}
    ins = (x.ap(), xr.ap(), wqkv.ap(), bqk_pt.ap(), bv.ap(), wm.ap(),
           bm4.ap(), w1.ap(), b1_pt.ap(), w2.ap(), b2.ap())
    with tile.TileContext(nc) as tc:
        build_body(tc, ins, outs)
    nc.compile()
    return nc


def make_in_maps(inputs):
    """inputs: dict from reference.setup_inputs() (numpy f32). Returns list of 8 in_maps."""
    bf = ml_dtypes.bfloat16
    x = np.asarray(inputs["x"], np.float32)
    ln1_g = np.asarray(inputs["ln1_g"], np.float32)
    ln1_b = np.asarray(inputs["ln1_b"], np.float32)
    Wqkv = np.asarray(inputs["Wqkv"], np.float32)
    bqkv = np.asarray(inputs["bqkv"], np.float32)
    Wm = np.asarray(inputs["Wm"], np.float32)
    bm = np.asarray(inputs["bm"], np.float32)
    ln2_g = np.asarray(inputs["ln2_g"], np.float32)
    ln2_b = np.asarray(inputs["ln2_b"], np.float32)
    W1 = np.asarray(inputs["W1"], np.float32)
    b1 = np.asarray(inputs["b1"], np.float32)
    W2 = np.asarray(inputs["W2"], np.float32)
    b2 = np.asarray(inputs["b2"], np.float32)

    Wqkv_eff = ln1_g[:, None] * Wqkv
    bqkv_eff = ln1_b @ Wqkv + bqkv
    W1_eff = ln2_g[:, None] * W1
    b1_eff = ln2_b @ W1 + b1

    DQ = 512
    in_maps = []
    for c in range(8):
        b = c // 4
        g = c % 4
        qcols = slice(DH * 2 * g, DH * 2 * g + 2 * DH)
        kcols = slice(DQ + DH * 2 * g, DQ + DH * 2 * g + 2 * DH)
        vcols = slice(2 * DQ + H2 * DV * g, 2 * DQ + H2 * DV * (g + 1))
        wqkv_c = np.concatenate(
            [Wqkv_eff[:, qcols], Wqkv_eff[:, kcols], Wqkv_eff[:, vcols]], axis=1)
        bq = bqkv_eff[qcols]
        bk = bqkv_eff[kcols]
        bv_c = bqkv_eff[vcols]
        bqk_pt = np.stack([bq, bk], axis=1)  # [128, 2]
        wm_c = Wm[H2 * DV * g:H2 * DV * (g + 1), :]
        rank = g
        xr = np.stack([x[b, QB * j + P * rank:QB * j + P * (rank + 1), :]
                       for j in range(NQB)])
        in_maps.append({
            "x": np.ascontiguousarray(x[b]),
            "xr": np.ascontiguousarray(xr),
            "wqkv": np.ascontiguousarray(wqkv_c.astype(bf)),
            "bqk_pt": np.ascontiguousarray(bqk_pt),
            "bv": np.ascontiguousarray(bv_c[None, :]),
            "wm": np.ascontiguousarray(wm_c.astype(bf)),
            "bm4": np.ascontiguousarray((bm / 4.0)[None, :].astype(np.float32)),
            "w1": np.ascontiguousarray(W1_eff.astype(bf)),
            "b1_pt": np.ascontiguousarray(b1_eff.reshape(EC, P).T),
            "w2": np.ascontiguousarray(W2.astype(bf)),
            "b2": np.ascontiguousarray(b2[None, :]),
        })
    return in_maps


def assemble_output(results):
    """results: list of 8 dicts with 'out' [512, 512]. Returns (2, 2048, 512)."""
    full = np.empty((2, N, D), np.float32)
    for c in range(8):
        b, rank = c // 4, c % 4
        o = results[c]["out"]
        for j in range(NQB):
            full[b, QB * j + P * rank:QB * j + P * (rank + 1), :] = \
                o[P * j:P * (j + 1), :]
    return full


_NC_CACHE = {}


def kernel(**inputs) -> np.ndarray:
    """Full-input entry point: shards across 8 NeuronCores, returns full output."""
    key = "nc8"
    if key not in _NC_CACHE:
        _NC_CACHE[key] = build_nc()
    nc = _NC_CACHE[key]
    in_maps = make_in_maps(inputs)
    res = bass_utils.run_bass_kernel_spmd(nc, in_maps, core_ids=list(range(8)))
    return assemble_output(res.results)
